# revision 37
# baseline (speedup 1.0000x reference)
"""Trainium2 Bass kernel for nn_MultiHeadAttention (S=2048, B=4, D=1024, H=16).

Sharding: 8 cores = 4 batches x 2 head-groups (8 heads each).
Each core computes, for its (batch b, head-group g):
  Q^T/K^T = Wslice @ x^T (dim-major), V = x @ Wslice^T (seq-major, + ones col)
  S^T = K_h @ Q_h^T  (keys on partitions), P^T = exp(S^T/8)  (ACT, bf16)
  ctx^T+sums = [V_h|1]^T @ P^T  (PSUM accum over key chunks)
  ctx = ctx_unnorm * (1/sums)   (DVE recip_approx_fast + DMA broadcast)
  out^T_partial = Wo_slice^T.T @ ctx^T
Host: out[:, b, :] = (partial[2b] + partial[2b+1]).T + bo

Schedule: scores+exp emitted one iteration ahead of the ctx matmuls (no
PE head-of-line block on ACT); per-head-pair windows carry the next
pair's K/Q projections (K fully + Q n-tile 0; later Q n-tiles projected
just-in-time inside their own window); out-projection overlapped into
the last head-pair's windows, gated per normalized q-tile.
"""

import sys

for _p in ("/opt/trn_rl_repo", "/opt/pypackages"):
    if _p not in sys.path:
        sys.path.append(_p)

import numpy as np
import ml_dtypes

import concourse.bacc as bacc
import concourse.tile as tile
from concourse import mybir
from concourse.bass_utils import run_bass_kernel_spmd

BF16 = ml_dtypes.bfloat16
FP32 = np.float32

D = 1024          # model dim
H_CORE = 8        # heads per core
DK = 64           # head dim
DH = H_CORE * DK  # 512 per-core head dims
N_CORES = 8

bf = mybir.dt.bfloat16
f32 = mybir.dt.float32


def build_program(S=2048, has_bias=False):
    """Build the per-core SPMD Bass program. Returns (nc, names)."""
    assert S % 512 == 0
    NSC = S // 128    # seq/key chunks of 128
    NQT = S // 512    # q tiles of 512
    NKD = D // 128    # 8 model-dim contraction chunks
    NKO = DH // 128   # 4 ctx contraction chunks
    NMO = D // 128    # 8 out-dim chunks
    NHP = H_CORE // 2 # 4 head pairs

    nc = bacc.Bacc(None, target_bir_lowering=False)

    xq_d = nc.dram_tensor("xq", [D, S], bf, kind="ExternalInput")
    xk_d = nc.dram_tensor("xk", [D, S], bf, kind="ExternalInput")
    xv_d = nc.dram_tensor("xv", [D, S], bf, kind="ExternalInput")
    # weights pre-tiled on host to [128, nchunks*cols] partition-major
    wq_d = nc.dram_tensor("wq", [128, NKD * DH], bf, kind="ExternalInput")
    wk_d = nc.dram_tensor("wk", [128, NKD * DH], bf, kind="ExternalInput")
    wv_d = nc.dram_tensor("wv", [128, NKD * DH], bf, kind="ExternalInput")
    wo_d = nc.dram_tensor("wo", [128, NKO * D], bf, kind="ExternalInput")
    bq_d = nc.dram_tensor("bq", [1, DH], bf, kind="ExternalInput")
    bk_d = nc.dram_tensor("bk", [1, DH], bf, kind="ExternalInput")
    bv_d = nc.dram_tensor("bv", [1, DH], bf, kind="ExternalInput")
    out_d = nc.dram_tensor("out", [D, S], f32, kind="ExternalOutput")

    with tile.TileContext(nc) as tc:
        _build_body(nc, tc, S, NSC, NQT, NKD, NKO, NMO, NHP,
                    xq_d, xk_d, xv_d, wq_d, wk_d, wv_d, wo_d,
                    bq_d, bk_d, bv_d, out_d, has_bias=has_bias)
    nc.compile()
    return nc


def _build_body(nc, tc, S, NSC, NQT, NKD, NKO, NMO, NHP,
                xq_d, xk_d, xv_d, wq_d, wk_d, wv_d, wo_d,
                bq_d, bk_d, bv_d, out_d, has_bias=False):
    from contextlib import ExitStack
    ctx = ExitStack()
    with ctx:
        consts = ctx.enter_context(tc.tile_pool(name="consts", bufs=1))
        wpool = ctx.enter_context(tc.tile_pool(name="wpool", bufs=1))
        xqk_pool = ctx.enter_context(tc.tile_pool(name="xqk", bufs=1))
        xv_pool = ctx.enter_context(tc.tile_pool(name="xvp", bufs=NKD))
        p_pool = ctx.enter_context(tc.tile_pool(name="pp", bufs=5 if not has_bias else 3))
        vpool = ctx.enter_context(tc.tile_pool(name="vpool", bufs=1))
        qk_pool = ctx.enter_context(tc.tile_pool(name="qkT", bufs=2))
        ctx_pool = ctx.enter_context(tc.tile_pool(name="ctxp", bufs=1))
        norm_pool = ctx.enter_context(tc.tile_pool(name="normp", bufs=3))
        dram_pool = ctx.enter_context(tc.tile_pool(name="dramp", bufs=2, space="DRAM"))
        psum_s = ctx.enter_context(tc.tile_pool(name="psum_s", bufs=2, space="PSUM"))
        psum_c = ctx.enter_context(tc.tile_pool(name="psum_c", bufs=2, space="PSUM"))
        psum_ctx = ctx.enter_context(tc.tile_pool(name="psum_ctx", bufs=2, space="PSUM"))

        # ---- constants / weights -------------------------------------------
        ones_row = consts.tile([1, 512], bf)
        nc.vector.memset(ones_row[:], 1.0)

        wq_sb = wpool.tile([128, NKD, DH], bf, tag="wq")
        wk_sb = wpool.tile([128, NKD, DH], bf, tag="wk")
        wv_sb = wpool.tile([128, NKD, DH], bf, tag="wv")
        bq_sb = bk_sb = bv_sb = None
        if has_bias:
            bq_sb = consts.tile([1, DH], bf, tag="bq")
            bk_sb = consts.tile([1, DH], bf, tag="bk")
            bv_sb = consts.tile([1, DH], bf, tag="bv")

        # load order: K-proj inputs first so the PE starts earliest, then Q,
        # then V. Inputs land in 512-column n-blocks so consumers start
        # before the whole input has arrived.
        wk_r = wk_d[:].rearrange("p (c n) -> p c n", c=NKD)
        wq_r = wq_d[:].rearrange("p (c n) -> p c n", c=NKD)
        nc.sync.dma_start(out=wk_sb[:, 0:NKD // 2], in_=wk_r[:, 0:NKD // 2])
        nc.sync.dma_start(out=wk_sb[:, NKD // 2:NKD], in_=wk_r[:, NKD // 2:NKD])
        xq_tiles, xk_tiles, xv_tiles = [], [], []
        for k in range(NKD):
            xk_tiles.append(xqk_pool.tile([128, S], bf, name=f"xk{k}", tag=f"xk{k}"))
            xq_tiles.append(xqk_pool.tile([128, S], bf, name=f"xq{k}", tag=f"xq{k}"))
            xv_tiles.append(xv_pool.tile([128, S], bf, name=f"xv{k}", tag="xvt"))
        for k in range(NKD):
            nc.sync.dma_start(out=xk_tiles[k][:, 0:512],
                              in_=xk_d[128 * k:128 * (k + 1), 0:512])
        nc.sync.dma_start(out=wq_sb[:, 0:NKD // 2], in_=wq_r[:, 0:NKD // 2])
        nc.sync.dma_start(out=wq_sb[:, NKD // 2:NKD], in_=wq_r[:, NKD // 2:NKD])
        if has_bias:
            nc.sync.dma_start(out=bq_sb[:], in_=bq_d[:])
            nc.sync.dma_start(out=bk_sb[:], in_=bk_d[:])
            nc.sync.dma_start(out=bv_sb[:], in_=bv_d[:])
        for k in range(NKD):
            nc.sync.dma_start(out=xq_tiles[k][:, 0:512],
                              in_=xq_d[128 * k:128 * (k + 1), 0:512])
        nc.sync.dma_start(out=wv_sb[:], in_=wv_d[:].rearrange("p (c n) -> p c n", c=NKD))
        for k in range(NKD):
            nc.sync.dma_start(out=xv_tiles[k][:, 0:512],
                              in_=xv_d[128 * k:128 * (k + 1), 0:512])
        for n in range(1, NQT):
            cs = slice(512 * n, 512 * (n + 1))
            for k in range(NKD):
                nc.sync.dma_start(out=xk_tiles[k][:, cs],
                                  in_=xk_d[128 * k:128 * (k + 1), cs])
            for k in range(NKD):
                nc.sync.dma_start(out=xq_tiles[k][:, cs],
                                  in_=xq_d[128 * k:128 * (k + 1), cs])
            for k in range(NKD):
                nc.sync.dma_start(out=xv_tiles[k][:, cs],
                                  in_=xv_d[128 * k:128 * (k + 1), cs])

        v_sb = [None] * NSC

        def emit_v_chunk(s):
            ps = psum_c.tile([128, 512], f32, name=f"vps{s}", tag="ps512")
            for k in range(NKD):
                nc.tensor.matmul(
                    ps[:], xv_tiles[k][:, 128 * s:128 * (s + 1)], wv_sb[:, k, :],
                    start=(k == 0), stop=(not has_bias and k == NKD - 1))
            if has_bias:
                nc.tensor.matmul(ps[:], ones_row[0:1, 0:128], bv_sb[:],
                                 start=False, stop=True)
            vt = vpool.tile([128, H_CORE, DK + 1], bf, name=f"v{s}", tag=f"v{s}")
            nc.vector.tensor_copy(
                out=vt[:, :, 0:DK],
                in_=ps[:].rearrange("p (h d) -> p h d", h=H_CORE))
            nc.gpsimd.memset(vt[:, :, DK:DK + 1], 1.0)
            v_sb[s] = vt

        _proj_ps = {}

        def emit_proj_ntile(args, phase=None):
            # phase None: whole tile; phase 0/1: first/second half of the
            # contraction, so pipelined bursts stay short and don't starve ACT
            w_sb, b_sb, x_tiles, dst, hp, n = args
            key = (id(dst), n)
            if phase == 1 and key in _proj_ps:
                ps = _proj_ps.pop(key)
                ks = range(NKD // 2, NKD)
            else:
                ps = psum_c.tile([128, 512], f32, name=f"pj{hp}{n}", tag="ps512")
                ks = range(NKD) if phase is None else range(NKD // 2)
            for k in ks:
                nc.tensor.matmul(
                    ps[:],
                    w_sb[:, k, 128 * hp:128 * (hp + 1)],
                    x_tiles[k][:, 512 * n:512 * (n + 1)],
                    start=(k == 0), stop=(not has_bias and k == NKD - 1))
            if phase == 0:
                _proj_ps[key] = ps
                return
            if has_bias:
                nc.tensor.matmul(
                    ps[:], b_sb[0:1, 128 * hp:128 * (hp + 1)],
                    ones_row[0:1, 0:512],
                    start=False, stop=True)
            nc.vector.tensor_copy(out=dst[:, 512 * n:512 * (n + 1)], in_=ps[:])

        ctx_sb = [ctx_pool.tile([128, S], bf, name=f"ctx{hp}", tag=f"ctx{hp}")
                  for hp in range(NHP)]

        wo_t = {}
        out_i = [0]

        def emit_out_group(mo, nt):
            i_t = out_i[0]
            out_i[0] += 1
            ps = psum_c.tile([128, 512], f32, name=f"ops{mo}{nt}", tag="ps512")
            for i in range(NKO):
                nc.tensor.matmul(
                    ps[:],
                    wo_t[i][:, 128 * mo:128 * (mo + 1)],
                    ctx_sb[i][:, 512 * nt:512 * (nt + 1)],
                    start=(i == 0), stop=(i == NKO - 1))
            ot = xqk_pool.tile([128, 512], f32, name=f"o{mo}{nt}",
                               tag=f"xk{i_t % 8}")
            nc.vector.tensor_copy(out=ot[:], in_=ps[:])
            dst = out_d[128 * mo:128 * (mo + 1), 512 * nt:512 * (nt + 1)]
            nc.sync.dma_start(out=dst, in_=ot[:])

        # qT/kT tiles allocated one hp ahead (projections are pipelined into
        # the previous head pair's attention loop)
        qk_tiles = {}

        def get_qk(hp):
            if hp not in qk_tiles:
                qT = qk_pool.tile([128, S], bf, name=f"qT{hp}", tag="qT")
                kT = qk_pool.tile([128, S], bf, name=f"kT{hp}", tag="kT")
                qk_tiles[hp] = (qT, kT)
            return qk_tiles[hp]

        # out-proj groups emitted inside hp=NHP-1's loop once their q-tile's
        # ctx is normalized for all head pairs
        out_queue = []

        # ---- per head pair: attention (with pipelined proj of hp+1) --------
        for hp in range(NHP):
            qT, kT = get_qk(hp)
            if hp == 0:
                emit_proj_ntile((wk_sb, bk_sb, xk_tiles, kT, 0, 0))
            if hp == NHP - 1:
                # stage Wo early for the overlapped output projection
                for kc in range(NKO):
                    t = wpool.tile([128, D], bf, name=f"wo{kc}", tag=f"wo{kc}",
                                   bufs=1)
                    nc.sync.dma_start(out=t[:], in_=wo_d[:, D * kc:D * (kc + 1)])
                    wo_t[kc] = t
            # work queue of next-hp projection tiles to sprinkle into qt2/qt3:
            # all of kT (every q-tile reads all keys) plus qT's first n-tile.
            # qT n>=1 of the NEXT hp is emitted just-in-time inside its own
            # window (jit_q), relieving this window's PE load.
            pending = []
            if hp + 1 < NHP:
                nqT, nkT = get_qk(hp + 1)
                for n in range(NQT):
                    pending.append((wk_sb, bk_sb, xk_tiles, nkT, hp + 1, n))
                pending.append((wq_sb, bq_sb, xq_tiles, nqT, hp + 1, 0))
            jit_q = []
            if hp > 0:
                for n in range(1, NQT):
                    jit_q.append((wq_sb, bq_sb, xq_tiles, qT, hp, n))
            if hp == 0:
                # n>=2 via the jit slots; n=1 as a burst at the end of qt0's
                # kc loop (qt0's slots are fully taken by V-chunk builds)
                for n in range(2, NQT):
                    jit_q.append((wq_sb, bq_sb, xq_tiles, qT, 0, n))

            cu_store = {}
            for qt in range(NQT):
                if hp == 0 and qt == 0:
                    emit_proj_ntile((wq_sb, bq_sb, xq_tiles, qT, 0, 0))
                cps0 = psum_ctx.tile([128, 512], f32, name=f"c0_{hp}_{qt}", tag="cps")
                cps1 = psum_ctx.tile([128, 512], f32, name=f"c1_{hp}_{qt}", tag="cps")
                pts = {}

                def emit_scores_exp(kc, qT=None, kT=None, hp=hp, qt=qt):
                    # spare-slot work for this iteration, then scores + exp.
                    # Emitted one iteration ahead of the ctx matmuls so the
                    # PE never head-of-line blocks on the current exp.
                    if hp == 0 and qt == 0:
                        if kc % 4 == 3 and kc < NSC - 1:
                            emit_proj_ntile((wk_sb, bk_sb, xk_tiles, kT, 0,
                                             (kc + 1) // 4))
                        emit_v_chunk(kc)
                    elif pending and qt >= 2 and kc % 4 == 0:
                        emit_proj_ntile(pending[0], phase=0)
                    elif pending and qt >= 2 and kc % 4 == 2:
                        emit_proj_ntile(pending.pop(0), phase=1)
                    elif jit_q and qt + 1 == jit_q[0][-1] and kc % 4 == 1:
                        emit_proj_ntile(jit_q[0], phase=0)
                    elif jit_q and qt + 1 == jit_q[0][-1] and kc % 4 == 3:
                        emit_proj_ntile(jit_q.pop(0), phase=1)
                    elif out_queue and kc % 2 == 1:
                        emit_out_group(*out_queue.pop(0))
                    st = psum_s.tile([128, 1024], f32, name=f"st{hp}{qt}{kc}", tag="st")
                    for j in range(2):  # head within pair
                        r0 = 64 * j
                        nc.tensor.matmul(
                            st[:, 512 * j:512 * (j + 1)],
                            kT[r0:r0 + 64, 128 * kc:128 * (kc + 1)],
                            qT[r0:r0 + 64, 512 * qt:512 * (qt + 1)],
                            start=True, stop=True,
                            tile_position=(r0, 0))
                    pt = p_pool.tile([128, 1024], bf, name=f"p{hp}{qt}{kc}",
                                     tag="pp")
                    nc.scalar.activation(out=pt[:], in_=st[:],
                                         func=mybir.ActivationFunctionType.Exp,
                                         scale=0.125)
                    pts[kc] = pt

                emit_scores_exp(0, qT=qT, kT=kT)
                for kc in range(NSC):
                    if kc + 1 < NSC:
                        emit_scores_exp(kc + 1, qT=qT, kT=kT)
                    pt = pts.pop(kc)
                    for j, cps in enumerate((cps0, cps1)):
                        nc.tensor.matmul(
                            cps[0:DK + 1, :],
                            v_sb[kc][:, 2 * hp + j, :],
                            pt[:, 512 * j:512 * (j + 1)],
                            start=(kc == 0), stop=(kc == NSC - 1))
                if hp == 0 and qt == 0 and NQT > 1:
                    emit_proj_ntile((wq_sb, bq_sb, xq_tiles, qT, 0, 1))
                # evict unnormalized ctx + sums; per-q-tile normalization so
                # hp3's out-proj groups unlock one q-tile at a time
                stage = norm_pool.tile([33, 512], f32, name=f"stage{hp}_{qt}",
                                       tag="stage", bufs=2)
                nc.gpsimd.memset(stage[:], 1.0)
                for j, cps in enumerate((cps0, cps1)):
                    nc.vector.tensor_copy(
                        out=stage[32 * j:32 * j + 1, :],
                        in_=cps[DK:DK + 1, :])
                    cu = norm_pool.tile([DK, 512], bf,
                                        name=f"cu{hp}{qt}{j}", tag="cu", bufs=5)
                    nc.vector.tensor_copy(out=cu[:], in_=cps[0:DK, :])
                    cu_store[(qt, j)] = cu
                recips = norm_pool.tile([33, 512], f32, name=f"rc{hp}_{qt}",
                                        tag="recips", bufs=2)
                nc.vector.reciprocal_approx_fast(out=recips[:], in_=stage[:])
                rdram = dram_pool.tile([2, 512], f32, name=f"rd{hp}{qt}", tag="rd")
                for j in range(2):
                    nc.sync.dma_start(
                        out=rdram[j:j + 1, :],
                        in_=recips[32 * j:32 * j + 1, :])
                for j in range(2):
                    bcast = norm_pool.tile(
                        [DK, 512], f32, name=f"b{hp}{qt}{j}",
                        tag="bcast", bufs=2)
                    nc.sync.dma_start(
                        out=bcast[:],
                        in_=rdram[j:j + 1, :].to_broadcast([DK, 512]))
                    nc.vector.tensor_mul(
                        out=ctx_sb[hp][64 * j:64 * (j + 1),
                                       512 * qt:512 * (qt + 1)],
                        in0=cu_store[(qt, j)][:], in1=bcast[:])
                if hp == NHP - 1:
                    # this q-tile's ctx now final across all head pairs
                    for mo in range(NMO):
                        out_queue.append((mo, qt))
            # small-S configs: drain work not absorbed by qt>=2 slots
            for args in pending:
                emit_proj_ntile(args)

        # ---- output projection remainder ----------------------------------
        for mo, nt in out_queue:
            emit_out_group(mo, nt)


# ----------------------------------------------------------------------------
# host side
# ----------------------------------------------------------------------------

def _tile_w(wT, nchunks):
    """[K, M] -> [128, nchunks*M] partition-major bf16."""
    K, M = wT.shape
    assert K == nchunks * 128
    return np.ascontiguousarray(
        wT.reshape(nchunks, 128, M).transpose(1, 0, 2).reshape(128, nchunks * M)
    ).astype(BF16)


def make_in_maps(query, key, value, Wq, bq, Wk, bk, Wv, bv, Wo, S=2048):
    in_maps = []
    for c in range(N_CORES):
        b, g = divmod(c, 2)
        hd = slice(DH * g, DH * (g + 1))
        m = {
            "xq": np.ascontiguousarray(query[:, b, :].T).astype(BF16),
            "xk": np.ascontiguousarray(key[:, b, :].T).astype(BF16),
            "xv": np.ascontiguousarray(value[:, b, :].T).astype(BF16),
            "wq": _tile_w(np.ascontiguousarray(Wq[hd, :].T), D // 128),
            "wk": _tile_w(np.ascontiguousarray(Wk[hd, :].T), D // 128),
            "wv": _tile_w(np.ascontiguousarray(Wv[hd, :].T), D // 128),
            "wo": _tile_w(np.ascontiguousarray(Wo[:, hd].T), DH // 128),
            "bq": bq[hd].reshape(1, DH).astype(BF16),
            "bk": bk[hd].reshape(1, DH).astype(BF16),
            "bv": bv[hd].reshape(1, DH).astype(BF16),
        }
        in_maps.append(m)
    return in_maps


def combine_outputs(results, bo, S=2048):
    out = np.empty((S, 4, D), np.float32)
    for b in range(4):
        pT = results[2 * b]["out"] + results[2 * b + 1]["out"]  # [D, S]
        out[:, b, :] = pT.T
    out += bo.astype(np.float32)[None, None, :]
    return out


_NC_CACHE = {}


def get_program(S=2048, has_bias=False):
    key = (S, has_bias)
    if key not in _NC_CACHE:
        _NC_CACHE[key] = build_program(S, has_bias=has_bias)
    return _NC_CACHE[key]


def kernel(query, key, value, Wq, bq, Wk, bk, Wv, bv, Wo, bo, **_):
    query = np.asarray(query, np.float32)
    key = np.asarray(key, np.float32)
    value = np.asarray(value, np.float32)
    S = query.shape[0]
    has_bias = any(float(np.abs(np.asarray(b)).max()) != 0.0
                   for b in (bq, bk, bv))
    nc = get_program(S, has_bias=has_bias)
    in_maps = make_in_maps(query, key, value,
                           np.asarray(Wq), np.asarray(bq),
                           np.asarray(Wk), np.asarray(bk),
                           np.asarray(Wv), np.asarray(bv),
                           np.asarray(Wo), S=S)
    res = run_bass_kernel_spmd(nc, in_maps, core_ids=list(range(N_CORES)))
    return combine_outputs(res.results, np.asarray(bo), S=S)



# revision 40
# speedup vs baseline: 1.0073x; 1.0073x over previous
"""Trainium2 Bass kernel for nn_MultiHeadAttention (S=2048, B=4, D=1024, H=16).

Sharding: 8 cores = 4 batches x 2 head-groups (8 heads each).
Each core computes, for its (batch b, head-group g):
  Q^T/K^T = Wslice @ x^T (dim-major), V = x @ Wslice^T (seq-major, + ones col)
  S^T = K_h @ Q_h^T  (keys on partitions), P^T = exp(S^T/8)  (ACT, bf16)
  ctx^T+sums = [V_h|1]^T @ P^T  (PSUM accum over key chunks)
  ctx = ctx_unnorm * (1/sums)   (DVE recip_approx_fast + DMA broadcast)
  out^T_partial = Wo_slice^T.T @ ctx^T
Host: out[:, b, :] = (partial[2b] + partial[2b+1]).T + bo

Schedule: scores+exp emitted one iteration ahead of the ctx matmuls (no
PE head-of-line block on ACT); per-head-pair windows carry the next
pair's K/Q projections (K fully + Q n-tile 0; later Q n-tiles projected
just-in-time inside their own window); out-projection overlapped into
the last head-pair's windows, gated per normalized q-tile.
"""

import sys

for _p in ("/opt/trn_rl_repo", "/opt/pypackages"):
    if _p not in sys.path:
        sys.path.append(_p)

import numpy as np
import ml_dtypes

import concourse.bacc as bacc
import concourse.tile as tile
from concourse import mybir
from concourse.bass_utils import run_bass_kernel_spmd

BF16 = ml_dtypes.bfloat16
FP32 = np.float32

D = 1024          # model dim
H_CORE = 8        # heads per core
DK = 64           # head dim
DH = H_CORE * DK  # 512 per-core head dims
N_CORES = 8

bf = mybir.dt.bfloat16
f32 = mybir.dt.float32


def build_program(S=2048, has_bias=False):
    """Build the per-core SPMD Bass program. Returns (nc, names)."""
    assert S % 512 == 0
    NSC = S // 128    # seq/key chunks of 128
    NQT = S // 512    # q tiles of 512
    NKD = D // 128    # 8 model-dim contraction chunks
    NKO = DH // 128   # 4 ctx contraction chunks
    NMO = D // 128    # 8 out-dim chunks
    NHP = H_CORE // 2 # 4 head pairs

    nc = bacc.Bacc(None, target_bir_lowering=False)

    xq_d = nc.dram_tensor("xq", [D, S], bf, kind="ExternalInput")
    xk_d = nc.dram_tensor("xk", [D, S], bf, kind="ExternalInput")
    xv_d = nc.dram_tensor("xv", [D, S], bf, kind="ExternalInput")
    # weights pre-tiled on host to [128, nchunks*cols] partition-major
    wq_d = nc.dram_tensor("wq", [128, NKD * DH], bf, kind="ExternalInput")
    wk_d = nc.dram_tensor("wk", [128, NKD * DH], bf, kind="ExternalInput")
    wv_d = nc.dram_tensor("wv", [128, NKD * DH], bf, kind="ExternalInput")
    wo_d = nc.dram_tensor("wo", [128, NKO * D], bf, kind="ExternalInput")
    bq_d = nc.dram_tensor("bq", [1, DH], bf, kind="ExternalInput")
    bk_d = nc.dram_tensor("bk", [1, DH], bf, kind="ExternalInput")
    bv_d = nc.dram_tensor("bv", [1, DH], bf, kind="ExternalInput")
    out_d = nc.dram_tensor("out", [D, S], f32, kind="ExternalOutput")

    with tile.TileContext(nc) as tc:
        _build_body(nc, tc, S, NSC, NQT, NKD, NKO, NMO, NHP,
                    xq_d, xk_d, xv_d, wq_d, wk_d, wv_d, wo_d,
                    bq_d, bk_d, bv_d, out_d, has_bias=has_bias)
    nc.compile()
    return nc


def _build_body(nc, tc, S, NSC, NQT, NKD, NKO, NMO, NHP,
                xq_d, xk_d, xv_d, wq_d, wk_d, wv_d, wo_d,
                bq_d, bk_d, bv_d, out_d, has_bias=False):
    from contextlib import ExitStack
    ctx = ExitStack()
    with ctx:
        consts = ctx.enter_context(tc.tile_pool(name="consts", bufs=1))
        wpool = ctx.enter_context(tc.tile_pool(name="wpool", bufs=1))
        xqk_pool = ctx.enter_context(tc.tile_pool(name="xqk", bufs=1))
        xv_pool = ctx.enter_context(tc.tile_pool(name="xvp", bufs=NKD))
        p_pool = ctx.enter_context(tc.tile_pool(name="pp", bufs=5 if not has_bias else 3))
        vpool = ctx.enter_context(tc.tile_pool(name="vpool", bufs=1))
        qk_pool = ctx.enter_context(tc.tile_pool(name="qkT", bufs=2))
        ctx_pool = ctx.enter_context(tc.tile_pool(name="ctxp", bufs=1))
        norm_pool = ctx.enter_context(tc.tile_pool(name="normp", bufs=3))
        dram_pool = ctx.enter_context(tc.tile_pool(name="dramp", bufs=2, space="DRAM"))
        psum_s = ctx.enter_context(tc.tile_pool(name="psum_s", bufs=2, space="PSUM"))
        psum_c = ctx.enter_context(tc.tile_pool(name="psum_c", bufs=2, space="PSUM"))
        psum_ctx = ctx.enter_context(tc.tile_pool(name="psum_ctx", bufs=2, space="PSUM"))

        # ---- constants / weights -------------------------------------------
        ones_row = consts.tile([1, 512], bf)
        nc.vector.memset(ones_row[:], 1.0)

        # HAM warm-up: keep the PE busy through the input-DMA window so the
        # clock gate is already at 8/8 when the first real matmuls issue
        warm = consts.tile([128, 512], bf, tag="warm")
        nc.vector.memset(warm[:], 0.0)
        wsc = consts.tile([128, 512], f32, tag="wsc")
        wps = psum_c.tile([128, 512], f32, name="warmps", tag="ps512")
        for i in range(40):
            nc.tensor.matmul(wps[:], warm[:, 0:128], warm[:],
                             start=(i == 0), stop=(i == 39))
        nc.vector.tensor_copy(out=wsc[:], in_=wps[:])

        wq_sb = wpool.tile([128, NKD, DH], bf, tag="wq")
        wk_sb = wpool.tile([128, NKD, DH], bf, tag="wk")
        wv_sb = wpool.tile([128, NKD, DH], bf, tag="wv")
        bq_sb = bk_sb = bv_sb = None
        if has_bias:
            bq_sb = consts.tile([1, DH], bf, tag="bq")
            bk_sb = consts.tile([1, DH], bf, tag="bk")
            bv_sb = consts.tile([1, DH], bf, tag="bv")

        # load order: K-proj inputs first so the PE starts earliest, then Q,
        # then V. Inputs land in 512-column n-blocks so consumers start
        # before the whole input has arrived.
        wk_r = wk_d[:].rearrange("p (c n) -> p c n", c=NKD)
        wq_r = wq_d[:].rearrange("p (c n) -> p c n", c=NKD)
        nc.sync.dma_start(out=wk_sb[:, 0:NKD // 2], in_=wk_r[:, 0:NKD // 2])
        nc.sync.dma_start(out=wk_sb[:, NKD // 2:NKD], in_=wk_r[:, NKD // 2:NKD])
        xq_tiles, xk_tiles, xv_tiles = [], [], []
        for k in range(NKD):
            xk_tiles.append(xqk_pool.tile([128, S], bf, name=f"xk{k}", tag=f"xk{k}"))
            xq_tiles.append(xqk_pool.tile([128, S], bf, name=f"xq{k}", tag=f"xq{k}"))
            xv_tiles.append(xv_pool.tile([128, S], bf, name=f"xv{k}", tag="xvt"))
        for k in range(NKD):
            nc.sync.dma_start(out=xk_tiles[k][:, 0:512],
                              in_=xk_d[128 * k:128 * (k + 1), 0:512])
        nc.sync.dma_start(out=wq_sb[:, 0:NKD // 2], in_=wq_r[:, 0:NKD // 2])
        nc.sync.dma_start(out=wq_sb[:, NKD // 2:NKD], in_=wq_r[:, NKD // 2:NKD])
        if has_bias:
            nc.sync.dma_start(out=bq_sb[:], in_=bq_d[:])
            nc.sync.dma_start(out=bk_sb[:], in_=bk_d[:])
            nc.sync.dma_start(out=bv_sb[:], in_=bv_d[:])
        for k in range(NKD):
            nc.sync.dma_start(out=xq_tiles[k][:, 0:512],
                              in_=xq_d[128 * k:128 * (k + 1), 0:512])
        nc.sync.dma_start(out=wv_sb[:], in_=wv_d[:].rearrange("p (c n) -> p c n", c=NKD))
        for k in range(NKD):
            nc.sync.dma_start(out=xv_tiles[k][:, 0:512],
                              in_=xv_d[128 * k:128 * (k + 1), 0:512])
        for n in range(1, NQT):
            cs = slice(512 * n, 512 * (n + 1))
            for k in range(NKD):
                nc.sync.dma_start(out=xk_tiles[k][:, cs],
                                  in_=xk_d[128 * k:128 * (k + 1), cs])
            for k in range(NKD):
                nc.sync.dma_start(out=xq_tiles[k][:, cs],
                                  in_=xq_d[128 * k:128 * (k + 1), cs])
            for k in range(NKD):
                nc.sync.dma_start(out=xv_tiles[k][:, cs],
                                  in_=xv_d[128 * k:128 * (k + 1), cs])

        v_sb = [None] * NSC

        def emit_v_chunk(s):
            ps = psum_c.tile([128, 512], f32, name=f"vps{s}", tag="ps512")
            for k in range(NKD):
                nc.tensor.matmul(
                    ps[:], xv_tiles[k][:, 128 * s:128 * (s + 1)], wv_sb[:, k, :],
                    start=(k == 0), stop=(not has_bias and k == NKD - 1))
            if has_bias:
                nc.tensor.matmul(ps[:], ones_row[0:1, 0:128], bv_sb[:],
                                 start=False, stop=True)
            vt = vpool.tile([128, H_CORE, DK + 1], bf, name=f"v{s}", tag=f"v{s}")
            nc.vector.tensor_copy(
                out=vt[:, :, 0:DK],
                in_=ps[:].rearrange("p (h d) -> p h d", h=H_CORE))
            nc.gpsimd.memset(vt[:, :, DK:DK + 1], 1.0)
            v_sb[s] = vt

        _proj_ps = {}

        def emit_proj_ntile(args, phase=None):
            # phase None: whole tile; phase 0/1: first/second half of the
            # contraction, so pipelined bursts stay short and don't starve ACT
            w_sb, b_sb, x_tiles, dst, hp, n = args
            key = (id(dst), n)
            if phase == 1 and key in _proj_ps:
                ps = _proj_ps.pop(key)
                ks = range(NKD // 2, NKD)
            else:
                ps = psum_c.tile([128, 512], f32, name=f"pj{hp}{n}", tag="ps512")
                ks = range(NKD) if phase is None else range(NKD // 2)
            for k in ks:
                nc.tensor.matmul(
                    ps[:],
                    w_sb[:, k, 128 * hp:128 * (hp + 1)],
                    x_tiles[k][:, 512 * n:512 * (n + 1)],
                    start=(k == 0), stop=(not has_bias and k == NKD - 1))
            if phase == 0:
                _proj_ps[key] = ps
                return
            if has_bias:
                nc.tensor.matmul(
                    ps[:], b_sb[0:1, 128 * hp:128 * (hp + 1)],
                    ones_row[0:1, 0:512],
                    start=False, stop=True)
            nc.vector.tensor_copy(out=dst[:, 512 * n:512 * (n + 1)], in_=ps[:])

        ctx_sb = [ctx_pool.tile([128, S], bf, name=f"ctx{hp}", tag=f"ctx{hp}")
                  for hp in range(NHP)]

        wo_t = {}
        out_i = [0]

        def emit_out_group(mo, nt):
            i_t = out_i[0]
            out_i[0] += 1
            ps = psum_c.tile([128, 512], f32, name=f"ops{mo}{nt}", tag="ps512")
            for i in range(NKO):
                nc.tensor.matmul(
                    ps[:],
                    wo_t[i][:, 128 * mo:128 * (mo + 1)],
                    ctx_sb[i][:, 512 * nt:512 * (nt + 1)],
                    start=(i == 0), stop=(i == NKO - 1))
            ot = xqk_pool.tile([128, 512], f32, name=f"o{mo}{nt}",
                               tag=f"xk{i_t % 8}")
            nc.vector.tensor_copy(out=ot[:], in_=ps[:])
            dst = out_d[128 * mo:128 * (mo + 1), 512 * nt:512 * (nt + 1)]
            nc.sync.dma_start(out=dst, in_=ot[:])

        # qT/kT tiles allocated one hp ahead (projections are pipelined into
        # the previous head pair's attention loop)
        qk_tiles = {}

        def get_qk(hp):
            if hp not in qk_tiles:
                qT = qk_pool.tile([128, S], bf, name=f"qT{hp}", tag="qT")
                kT = qk_pool.tile([128, S], bf, name=f"kT{hp}", tag="kT")
                qk_tiles[hp] = (qT, kT)
            return qk_tiles[hp]

        # out-proj groups emitted inside hp=NHP-1's loop once their q-tile's
        # ctx is normalized for all head pairs
        out_queue = []

        # ---- per head pair: attention (with pipelined proj of hp+1) --------
        for hp in range(NHP):
            qT, kT = get_qk(hp)
            if hp == 0:
                emit_proj_ntile((wk_sb, bk_sb, xk_tiles, kT, 0, 0))
            if hp == NHP - 1:
                # stage Wo early for the overlapped output projection
                for kc in range(NKO):
                    t = wpool.tile([128, D], bf, name=f"wo{kc}", tag=f"wo{kc}",
                                   bufs=1)
                    nc.sync.dma_start(out=t[:], in_=wo_d[:, D * kc:D * (kc + 1)])
                    wo_t[kc] = t
            # work queue of next-hp projection tiles to sprinkle into qt2/qt3:
            # all of kT (every q-tile reads all keys) plus qT's first n-tile.
            # qT n>=1 of the NEXT hp is emitted just-in-time inside its own
            # window (jit_q), relieving this window's PE load.
            pending = []
            if hp + 1 < NHP:
                nqT, nkT = get_qk(hp + 1)
                for n in range(NQT):
                    pending.append((wk_sb, bk_sb, xk_tiles, nkT, hp + 1, n))
                pending.append((wq_sb, bq_sb, xq_tiles, nqT, hp + 1, 0))
            jit_q = []
            if hp > 0:
                for n in range(1, NQT):
                    jit_q.append((wq_sb, bq_sb, xq_tiles, qT, hp, n))
            if hp == 0:
                # n>=2 via the jit slots; n=1 as a burst at the end of qt0's
                # kc loop (qt0's slots are fully taken by V-chunk builds)
                for n in range(2, NQT):
                    jit_q.append((wq_sb, bq_sb, xq_tiles, qT, 0, n))

            cu_store = {}
            for qt in range(NQT):
                if hp == 0 and qt == 0:
                    emit_proj_ntile((wq_sb, bq_sb, xq_tiles, qT, 0, 0))
                cps0 = psum_ctx.tile([128, 512], f32, name=f"c0_{hp}_{qt}", tag="cps")
                cps1 = psum_ctx.tile([128, 512], f32, name=f"c1_{hp}_{qt}", tag="cps")
                pts = {}

                def emit_slot(kc, qT=None, kT=None, hp=hp, qt=qt):
                    # spare-slot work (projections / out-proj) for iteration
                    # kc. Emitted AFTER that iteration's scores+exp and
                    # before the previous iteration's ctx, so it runs while
                    # the exp completes instead of the PE stalling at ctx.
                    if hp == 0 and qt == 0:
                        if kc % 4 == 3 and kc < NSC - 1:
                            emit_proj_ntile((wk_sb, bk_sb, xk_tiles, kT, 0,
                                             (kc + 1) // 4))
                        emit_v_chunk(kc)
                    elif pending and qt >= 2 and kc % 4 == 0:
                        emit_proj_ntile(pending[0], phase=0)
                    elif pending and qt >= 2 and kc % 4 == 2:
                        emit_proj_ntile(pending.pop(0), phase=1)
                    elif jit_q and qt + 1 == jit_q[0][-1] and kc % 4 == 1:
                        emit_proj_ntile(jit_q[0], phase=0)
                    elif jit_q and qt + 1 == jit_q[0][-1] and kc % 4 == 3:
                        emit_proj_ntile(jit_q.pop(0), phase=1)
                    elif out_queue and kc % 2 == 1:
                        emit_out_group(*out_queue.pop(0))

                def emit_scores_exp(kc, qT=None, kT=None, hp=hp, qt=qt):
                    st = psum_s.tile([128, 1024], f32, name=f"st{hp}{qt}{kc}", tag="st")
                    for j in range(2):  # head within pair
                        r0 = 64 * j
                        nc.tensor.matmul(
                            st[:, 512 * j:512 * (j + 1)],
                            kT[r0:r0 + 64, 128 * kc:128 * (kc + 1)],
                            qT[r0:r0 + 64, 512 * qt:512 * (qt + 1)],
                            start=True, stop=True,
                            tile_position=(r0, 0))
                    pt = p_pool.tile([128, 1024], bf, name=f"p{hp}{qt}{kc}",
                                     tag="pp")
                    nc.scalar.activation(out=pt[:], in_=st[:],
                                         func=mybir.ActivationFunctionType.Exp,
                                         scale=0.125)
                    pts[kc] = pt

                emit_scores_exp(0, qT=qT, kT=kT)
                emit_slot(0, qT=qT, kT=kT)
                for kc in range(NSC):
                    if kc + 1 < NSC:
                        emit_scores_exp(kc + 1, qT=qT, kT=kT)
                        emit_slot(kc + 1, qT=qT, kT=kT)
                    pt = pts.pop(kc)
                    for j, cps in enumerate((cps0, cps1)):
                        nc.tensor.matmul(
                            cps[0:DK + 1, :],
                            v_sb[kc][:, 2 * hp + j, :],
                            pt[:, 512 * j:512 * (j + 1)],
                            start=(kc == 0), stop=(kc == NSC - 1))
                if hp == 0 and qt == 0 and NQT > 1:
                    emit_proj_ntile((wq_sb, bq_sb, xq_tiles, qT, 0, 1))
                # evict unnormalized ctx + sums; per-q-tile normalization so
                # hp3's out-proj groups unlock one q-tile at a time
                stage = norm_pool.tile([33, 512], f32, name=f"stage{hp}_{qt}",
                                       tag="stage", bufs=2)
                nc.gpsimd.memset(stage[:], 1.0)
                for j, cps in enumerate((cps0, cps1)):
                    nc.vector.tensor_copy(
                        out=stage[32 * j:32 * j + 1, :],
                        in_=cps[DK:DK + 1, :])
                    cu = norm_pool.tile([DK, 512], bf,
                                        name=f"cu{hp}{qt}{j}", tag="cu", bufs=5)
                    nc.vector.tensor_copy(out=cu[:], in_=cps[0:DK, :])
                    cu_store[(qt, j)] = cu
                recips = norm_pool.tile([33, 512], f32, name=f"rc{hp}_{qt}",
                                        tag="recips", bufs=2)
                nc.vector.reciprocal_approx_fast(out=recips[:], in_=stage[:])
                rdram = dram_pool.tile([2, 512], f32, name=f"rd{hp}{qt}", tag="rd")
                for j in range(2):
                    nc.sync.dma_start(
                        out=rdram[j:j + 1, :],
                        in_=recips[32 * j:32 * j + 1, :])
                for j in range(2):
                    bcast = norm_pool.tile(
                        [DK, 512], f32, name=f"b{hp}{qt}{j}",
                        tag="bcast", bufs=2)
                    nc.sync.dma_start(
                        out=bcast[:],
                        in_=rdram[j:j + 1, :].to_broadcast([DK, 512]))
                    nc.vector.tensor_mul(
                        out=ctx_sb[hp][64 * j:64 * (j + 1),
                                       512 * qt:512 * (qt + 1)],
                        in0=cu_store[(qt, j)][:], in1=bcast[:])
                if hp == NHP - 1:
                    # this q-tile's ctx now final across all head pairs
                    for mo in range(NMO):
                        out_queue.append((mo, qt))
            # small-S configs: drain work not absorbed by qt>=2 slots
            for args in pending:
                emit_proj_ntile(args)

        # ---- output projection remainder ----------------------------------
        for mo, nt in out_queue:
            emit_out_group(mo, nt)


# ----------------------------------------------------------------------------
# host side
# ----------------------------------------------------------------------------

def _tile_w(wT, nchunks):
    """[K, M] -> [128, nchunks*M] partition-major bf16."""
    K, M = wT.shape
    assert K == nchunks * 128
    return np.ascontiguousarray(
        wT.reshape(nchunks, 128, M).transpose(1, 0, 2).reshape(128, nchunks * M)
    ).astype(BF16)


def make_in_maps(query, key, value, Wq, bq, Wk, bk, Wv, bv, Wo, S=2048):
    in_maps = []
    for c in range(N_CORES):
        b, g = divmod(c, 2)
        hd = slice(DH * g, DH * (g + 1))
        m = {
            "xq": np.ascontiguousarray(query[:, b, :].T).astype(BF16),
            "xk": np.ascontiguousarray(key[:, b, :].T).astype(BF16),
            "xv": np.ascontiguousarray(value[:, b, :].T).astype(BF16),
            "wq": _tile_w(np.ascontiguousarray(Wq[hd, :].T), D // 128),
            "wk": _tile_w(np.ascontiguousarray(Wk[hd, :].T), D // 128),
            "wv": _tile_w(np.ascontiguousarray(Wv[hd, :].T), D // 128),
            "wo": _tile_w(np.ascontiguousarray(Wo[:, hd].T), DH // 128),
            "bq": bq[hd].reshape(1, DH).astype(BF16),
            "bk": bk[hd].reshape(1, DH).astype(BF16),
            "bv": bv[hd].reshape(1, DH).astype(BF16),
        }
        in_maps.append(m)
    return in_maps


def combine_outputs(results, bo, S=2048):
    out = np.empty((S, 4, D), np.float32)
    for b in range(4):
        pT = results[2 * b]["out"] + results[2 * b + 1]["out"]  # [D, S]
        out[:, b, :] = pT.T
    out += bo.astype(np.float32)[None, None, :]
    return out


_NC_CACHE = {}


def get_program(S=2048, has_bias=False):
    key = (S, has_bias)
    if key not in _NC_CACHE:
        _NC_CACHE[key] = build_program(S, has_bias=has_bias)
    return _NC_CACHE[key]


def kernel(query, key, value, Wq, bq, Wk, bk, Wv, bv, Wo, bo, **_):
    query = np.asarray(query, np.float32)
    key = np.asarray(key, np.float32)
    value = np.asarray(value, np.float32)
    S = query.shape[0]
    has_bias = any(float(np.abs(np.asarray(b)).max()) != 0.0
                   for b in (bq, bk, bv))
    nc = get_program(S, has_bias=has_bias)
    in_maps = make_in_maps(query, key, value,
                           np.asarray(Wq), np.asarray(bq),
                           np.asarray(Wk), np.asarray(bk),
                           np.asarray(Wv), np.asarray(bv),
                           np.asarray(Wo), S=S)
    res = run_bass_kernel_spmd(nc, in_maps, core_ids=list(range(N_CORES)))
    return combine_outputs(res.results, np.asarray(bo), S=S)



# revision 42
# speedup vs baseline: 1.0123x; 1.0050x over previous
"""Trainium2 Bass kernel for nn_MultiHeadAttention (S=2048, B=4, D=1024, H=16).

Sharding: 8 cores = 4 batches x 2 head-groups (8 heads each).
Each core computes, for its (batch b, head-group g):
  Q^T/K^T = Wslice @ x^T (dim-major), V = x @ Wslice^T (seq-major, + ones col)
  S^T = K_h @ Q_h^T  (keys on partitions), P^T = exp(S^T/8)  (ACT, bf16)
  ctx^T+sums = [V_h|1]^T @ P^T  (PSUM accum over key chunks)
  ctx = ctx_unnorm * (1/sums)   (DVE recip_approx_fast + DMA broadcast)
  out^T_partial = Wo_slice^T.T @ ctx^T
Host: out[:, b, :] = (partial[2b] + partial[2b+1]).T + bo

Schedule: scores+exp emitted one iteration ahead of the ctx matmuls (no
PE head-of-line block on ACT); per-head-pair windows carry the next
pair's K/Q projections (K fully + Q n-tile 0; later Q n-tiles projected
just-in-time inside their own window); out-projection overlapped into
the last head-pair's windows, gated per normalized q-tile.
"""

import sys

for _p in ("/opt/trn_rl_repo", "/opt/pypackages"):
    if _p not in sys.path:
        sys.path.append(_p)

import numpy as np
import ml_dtypes

import concourse.bacc as bacc
import concourse.tile as tile
from concourse import mybir
from concourse.bass_utils import run_bass_kernel_spmd

BF16 = ml_dtypes.bfloat16
FP32 = np.float32

D = 1024          # model dim
H_CORE = 8        # heads per core
DK = 64           # head dim
DH = H_CORE * DK  # 512 per-core head dims
N_CORES = 8

bf = mybir.dt.bfloat16
f32 = mybir.dt.float32


def build_program(S=2048, has_bias=False):
    """Build the per-core SPMD Bass program. Returns (nc, names)."""
    assert S % 512 == 0
    NSC = S // 128    # seq/key chunks of 128
    NQT = S // 512    # q tiles of 512
    NKD = D // 128    # 8 model-dim contraction chunks
    NKO = DH // 128   # 4 ctx contraction chunks
    NMO = D // 128    # 8 out-dim chunks
    NHP = H_CORE // 2 # 4 head pairs

    nc = bacc.Bacc(None, target_bir_lowering=False)

    xq_d = nc.dram_tensor("xq", [D, S], bf, kind="ExternalInput")
    xk_d = nc.dram_tensor("xk", [D, S], bf, kind="ExternalInput")
    xv_d = nc.dram_tensor("xv", [D, S], bf, kind="ExternalInput")
    # weights pre-tiled on host to [128, nchunks*cols] partition-major
    wq_d = nc.dram_tensor("wq", [128, NKD * DH], bf, kind="ExternalInput")
    wk_d = nc.dram_tensor("wk", [128, NKD * DH], bf, kind="ExternalInput")
    wv_d = nc.dram_tensor("wv", [128, NKD * DH], bf, kind="ExternalInput")
    wo_d = nc.dram_tensor("wo", [128, NKO * D], bf, kind="ExternalInput")
    bq_d = nc.dram_tensor("bq", [1, DH], bf, kind="ExternalInput")
    bk_d = nc.dram_tensor("bk", [1, DH], bf, kind="ExternalInput")
    bv_d = nc.dram_tensor("bv", [1, DH], bf, kind="ExternalInput")
    out_d = nc.dram_tensor("out", [D, S], f32, kind="ExternalOutput")

    with tile.TileContext(nc) as tc:
        _build_body(nc, tc, S, NSC, NQT, NKD, NKO, NMO, NHP,
                    xq_d, xk_d, xv_d, wq_d, wk_d, wv_d, wo_d,
                    bq_d, bk_d, bv_d, out_d, has_bias=has_bias)
    nc.compile()
    return nc


def _build_body(nc, tc, S, NSC, NQT, NKD, NKO, NMO, NHP,
                xq_d, xk_d, xv_d, wq_d, wk_d, wv_d, wo_d,
                bq_d, bk_d, bv_d, out_d, has_bias=False):
    from contextlib import ExitStack
    ctx = ExitStack()
    with ctx:
        consts = ctx.enter_context(tc.tile_pool(name="consts", bufs=1))
        wpool = ctx.enter_context(tc.tile_pool(name="wpool", bufs=1))
        xqk_pool = ctx.enter_context(tc.tile_pool(name="xqk", bufs=1))
        xv_pool = ctx.enter_context(tc.tile_pool(name="xvp", bufs=NKD))
        p_pool = ctx.enter_context(tc.tile_pool(name="pp", bufs=5 if not has_bias else 3))
        vpool = ctx.enter_context(tc.tile_pool(name="vpool", bufs=1))
        qk_pool = ctx.enter_context(tc.tile_pool(name="qkT", bufs=2))
        ctx_pool = ctx.enter_context(tc.tile_pool(name="ctxp", bufs=1))
        norm_pool = ctx.enter_context(tc.tile_pool(name="normp", bufs=3))
        dram_pool = ctx.enter_context(tc.tile_pool(name="dramp", bufs=2, space="DRAM"))
        psum_s = ctx.enter_context(tc.tile_pool(name="psum_s", bufs=2, space="PSUM"))
        psum_c = ctx.enter_context(tc.tile_pool(name="psum_c", bufs=2, space="PSUM"))
        psum_ctx = ctx.enter_context(tc.tile_pool(name="psum_ctx", bufs=2, space="PSUM"))

        # ---- constants / weights -------------------------------------------
        ones_row = consts.tile([1, 512], bf)
        nc.vector.memset(ones_row[:], 1.0)

        # HAM warm-up: keep the PE busy through the input-DMA window so the
        # clock gate is already at 8/8 when the first real matmuls issue
        warm = consts.tile([128, 512], bf, tag="warm")
        nc.vector.memset(warm[:], 0.0)
        ones33 = consts.tile([33, 64], f32, tag="ones33")
        nc.vector.memset(ones33[:], 1.0)
        wsc = consts.tile([128, 512], f32, tag="wsc")
        wps = psum_c.tile([128, 512], f32, name="warmps", tag="ps512")
        for i in range(40):
            nc.tensor.matmul(wps[:], warm[:, 0:128], warm[:],
                             start=(i == 0), stop=(i == 39))
        nc.vector.tensor_copy(out=wsc[:], in_=wps[:])

        wq_sb = wpool.tile([128, NKD, DH], bf, tag="wq")
        wk_sb = wpool.tile([128, NKD, DH], bf, tag="wk")
        wv_sb = wpool.tile([128, NKD, DH], bf, tag="wv")
        bq_sb = bk_sb = bv_sb = None
        if has_bias:
            bq_sb = consts.tile([1, DH], bf, tag="bq")
            bk_sb = consts.tile([1, DH], bf, tag="bk")
            bv_sb = consts.tile([1, DH], bf, tag="bv")

        # load order: K-proj inputs first so the PE starts earliest, then Q,
        # then V. Inputs land in 512-column n-blocks so consumers start
        # before the whole input has arrived.
        wk_r = wk_d[:].rearrange("p (c n) -> p c n", c=NKD)
        wq_r = wq_d[:].rearrange("p (c n) -> p c n", c=NKD)
        nc.sync.dma_start(out=wk_sb[:, 0:NKD // 2], in_=wk_r[:, 0:NKD // 2])
        nc.sync.dma_start(out=wk_sb[:, NKD // 2:NKD], in_=wk_r[:, NKD // 2:NKD])
        xq_tiles, xk_tiles, xv_tiles = [], [], []
        for k in range(NKD):
            xk_tiles.append(xqk_pool.tile([128, S], bf, name=f"xk{k}", tag=f"xk{k}"))
            xq_tiles.append(xqk_pool.tile([128, S], bf, name=f"xq{k}", tag=f"xq{k}"))
            xv_tiles.append(xv_pool.tile([128, S], bf, name=f"xv{k}", tag="xvt"))
        for k in range(NKD):
            nc.sync.dma_start(out=xk_tiles[k][:, 0:512],
                              in_=xk_d[128 * k:128 * (k + 1), 0:512])
        nc.sync.dma_start(out=wq_sb[:, 0:NKD // 2], in_=wq_r[:, 0:NKD // 2])
        nc.sync.dma_start(out=wq_sb[:, NKD // 2:NKD], in_=wq_r[:, NKD // 2:NKD])
        if has_bias:
            nc.sync.dma_start(out=bq_sb[:], in_=bq_d[:])
            nc.sync.dma_start(out=bk_sb[:], in_=bk_d[:])
            nc.sync.dma_start(out=bv_sb[:], in_=bv_d[:])
        for k in range(NKD):
            nc.sync.dma_start(out=xq_tiles[k][:, 0:512],
                              in_=xq_d[128 * k:128 * (k + 1), 0:512])
        nc.sync.dma_start(out=wv_sb[:], in_=wv_d[:].rearrange("p (c n) -> p c n", c=NKD))
        for k in range(NKD):
            nc.sync.dma_start(out=xv_tiles[k][:, 0:512],
                              in_=xv_d[128 * k:128 * (k + 1), 0:512])
        for n in range(1, NQT):
            cs = slice(512 * n, 512 * (n + 1))
            for k in range(NKD):
                nc.sync.dma_start(out=xk_tiles[k][:, cs],
                                  in_=xk_d[128 * k:128 * (k + 1), cs])
            for k in range(NKD):
                nc.sync.dma_start(out=xq_tiles[k][:, cs],
                                  in_=xq_d[128 * k:128 * (k + 1), cs])
            for k in range(NKD):
                nc.sync.dma_start(out=xv_tiles[k][:, cs],
                                  in_=xv_d[128 * k:128 * (k + 1), cs])

        v_sb = [None] * NSC

        def emit_v_chunk(s):
            ps = psum_c.tile([128, 512], f32, name=f"vps{s}", tag="ps512")
            for k in range(NKD):
                nc.tensor.matmul(
                    ps[:], xv_tiles[k][:, 128 * s:128 * (s + 1)], wv_sb[:, k, :],
                    start=(k == 0), stop=(not has_bias and k == NKD - 1))
            if has_bias:
                nc.tensor.matmul(ps[:], ones_row[0:1, 0:128], bv_sb[:],
                                 start=False, stop=True)
            vt = vpool.tile([128, H_CORE, DK + 1], bf, name=f"v{s}", tag=f"v{s}")
            nc.vector.tensor_copy(
                out=vt[:, :, 0:DK],
                in_=ps[:].rearrange("p (h d) -> p h d", h=H_CORE))
            nc.gpsimd.memset(vt[:, :, DK:DK + 1], 1.0)
            v_sb[s] = vt

        _proj_ps = {}

        def emit_proj_ntile(args, phase=None):
            # phase None: whole tile; phase 0/1: first/second half of the
            # contraction, so pipelined bursts stay short and don't starve ACT
            w_sb, b_sb, x_tiles, dst, hp, n = args
            key = (id(dst), n)
            if phase == 1 and key in _proj_ps:
                ps = _proj_ps.pop(key)
                ks = range(NKD // 2, NKD)
            else:
                ps = psum_c.tile([128, 512], f32, name=f"pj{hp}{n}", tag="ps512")
                ks = range(NKD) if phase is None else range(NKD // 2)
            for k in ks:
                nc.tensor.matmul(
                    ps[:],
                    w_sb[:, k, 128 * hp:128 * (hp + 1)],
                    x_tiles[k][:, 512 * n:512 * (n + 1)],
                    start=(k == 0), stop=(not has_bias and k == NKD - 1))
            if phase == 0:
                _proj_ps[key] = ps
                return
            if has_bias:
                nc.tensor.matmul(
                    ps[:], b_sb[0:1, 128 * hp:128 * (hp + 1)],
                    ones_row[0:1, 0:512],
                    start=False, stop=True)
            nc.vector.tensor_copy(out=dst[:, 512 * n:512 * (n + 1)], in_=ps[:])

        ctx_sb = [ctx_pool.tile([128, S], bf, name=f"ctx{hp}", tag=f"ctx{hp}")
                  for hp in range(NHP)]

        wo_t = {}
        out_i = [0]

        def emit_out_group(mo, nt):
            i_t = out_i[0]
            out_i[0] += 1
            ps = psum_c.tile([128, 512], f32, name=f"ops{mo}{nt}", tag="ps512")
            for i in range(NKO):
                nc.tensor.matmul(
                    ps[:],
                    wo_t[i][:, 128 * mo:128 * (mo + 1)],
                    ctx_sb[i][:, 512 * nt:512 * (nt + 1)],
                    start=(i == 0), stop=(i == NKO - 1))
            ot = xqk_pool.tile([128, 512], f32, name=f"o{mo}{nt}",
                               tag=f"xk{i_t % 8}")
            nc.vector.tensor_copy(out=ot[:], in_=ps[:])
            dst = out_d[128 * mo:128 * (mo + 1), 512 * nt:512 * (nt + 1)]
            nc.sync.dma_start(out=dst, in_=ot[:])

        # qT/kT tiles allocated one hp ahead (projections are pipelined into
        # the previous head pair's attention loop)
        qk_tiles = {}

        def get_qk(hp):
            if hp not in qk_tiles:
                qT = qk_pool.tile([128, S], bf, name=f"qT{hp}", tag="qT")
                kT = qk_pool.tile([128, S], bf, name=f"kT{hp}", tag="kT")
                qk_tiles[hp] = (qT, kT)
            return qk_tiles[hp]

        # out-proj groups emitted inside hp=NHP-1's loop once their q-tile's
        # ctx is normalized for all head pairs
        out_queue = []

        # ---- per head pair: attention (with pipelined proj of hp+1) --------
        for hp in range(NHP):
            qT, kT = get_qk(hp)
            if hp == 0:
                emit_proj_ntile((wk_sb, bk_sb, xk_tiles, kT, 0, 0))
            if hp == NHP - 1:
                # stage Wo early for the overlapped output projection
                for kc in range(NKO):
                    t = wpool.tile([128, D], bf, name=f"wo{kc}", tag=f"wo{kc}",
                                   bufs=1)
                    nc.sync.dma_start(out=t[:], in_=wo_d[:, D * kc:D * (kc + 1)])
                    wo_t[kc] = t
            # work queue of next-hp projection tiles to sprinkle into qt2/qt3:
            # all of kT (every q-tile reads all keys) plus qT's first n-tile.
            # qT n>=1 of the NEXT hp is emitted just-in-time inside its own
            # window (jit_q), relieving this window's PE load.
            pending = []
            if hp + 1 < NHP:
                nqT, nkT = get_qk(hp + 1)
                for n in range(NQT):
                    pending.append((wk_sb, bk_sb, xk_tiles, nkT, hp + 1, n))
                pending.append((wq_sb, bq_sb, xq_tiles, nqT, hp + 1, 0))
            jit_q = []
            if hp > 0:
                for n in range(1, NQT):
                    jit_q.append((wq_sb, bq_sb, xq_tiles, qT, hp, n))
            if hp == 0:
                # n>=2 via the jit slots; n=1 as a burst at the end of qt0's
                # kc loop (qt0's slots are fully taken by V-chunk builds)
                for n in range(2, NQT):
                    jit_q.append((wq_sb, bq_sb, xq_tiles, qT, 0, n))

            cu_store = {}
            for qt in range(NQT):
                if hp == 0 and qt == 0:
                    emit_proj_ntile((wq_sb, bq_sb, xq_tiles, qT, 0, 0))
                cps0 = psum_ctx.tile([128, 512], f32, name=f"c0_{hp}_{qt}", tag="cps")
                cps1 = psum_ctx.tile([128, 512], f32, name=f"c1_{hp}_{qt}", tag="cps")
                pts = {}

                def emit_slot(kc, qT=None, kT=None, hp=hp, qt=qt):
                    # spare-slot work (projections / out-proj) for iteration
                    # kc. Emitted AFTER that iteration's scores+exp and
                    # before the previous iteration's ctx, so it runs while
                    # the exp completes instead of the PE stalling at ctx.
                    if hp == 0 and qt == 0:
                        if kc % 4 == 3 and kc < NSC - 1:
                            emit_proj_ntile((wk_sb, bk_sb, xk_tiles, kT, 0,
                                             (kc + 1) // 4))
                        emit_v_chunk(kc)
                    elif pending and qt >= 2 and kc % 4 == 0:
                        emit_proj_ntile(pending[0], phase=0)
                    elif pending and qt >= 2 and kc % 4 == 2:
                        emit_proj_ntile(pending.pop(0), phase=1)
                    elif jit_q and qt + 1 == jit_q[0][-1] and kc % 4 == 1:
                        emit_proj_ntile(jit_q[0], phase=0)
                    elif jit_q and qt + 1 == jit_q[0][-1] and kc % 4 == 3:
                        emit_proj_ntile(jit_q.pop(0), phase=1)
                    elif out_queue and kc % 2 == 1:
                        emit_out_group(*out_queue.pop(0))

                def emit_scores_exp(kc, qT=None, kT=None, hp=hp, qt=qt):
                    st = psum_s.tile([128, 1024], f32, name=f"st{hp}{qt}{kc}", tag="st")
                    for j in range(2):  # head within pair
                        r0 = 64 * j
                        nc.tensor.matmul(
                            st[:, 512 * j:512 * (j + 1)],
                            kT[r0:r0 + 64, 128 * kc:128 * (kc + 1)],
                            qT[r0:r0 + 64, 512 * qt:512 * (qt + 1)],
                            start=True, stop=True,
                            tile_position=(r0, 0))
                    pt = p_pool.tile([128, 1024], bf, name=f"p{hp}{qt}{kc}",
                                     tag="pp")
                    nc.scalar.activation(out=pt[:], in_=st[:],
                                         func=mybir.ActivationFunctionType.Exp,
                                         scale=0.125)
                    pts[kc] = pt

                emit_scores_exp(0, qT=qT, kT=kT)
                emit_slot(0, qT=qT, kT=kT)
                for kc in range(NSC):
                    if kc + 1 < NSC:
                        emit_scores_exp(kc + 1, qT=qT, kT=kT)
                        emit_slot(kc + 1, qT=qT, kT=kT)
                    pt = pts.pop(kc)
                    for j, cps in enumerate((cps0, cps1)):
                        nc.tensor.matmul(
                            cps[0:DK + 1, :],
                            v_sb[kc][:, 2 * hp + j, :],
                            pt[:, 512 * j:512 * (j + 1)],
                            start=(kc == 0), stop=(kc == NSC - 1))
                if hp == 0 and qt == 0 and NQT > 1:
                    emit_proj_ntile((wq_sb, bq_sb, xq_tiles, qT, 0, 1))
                # evict unnormalized ctx + sums; per-q-tile normalization so
                # hp3's out-proj groups unlock one q-tile at a time
                stage = norm_pool.tile([33, 512], f32, name=f"stage{hp}_{qt}",
                                       tag="stage", bufs=2)
                nc.gpsimd.memset(stage[:], 1.0)
                for j, cps in enumerate((cps0, cps1)):
                    nc.vector.tensor_copy(
                        out=stage[32 * j:32 * j + 1, :],
                        in_=cps[DK:DK + 1, :])
                    cu = norm_pool.tile([DK, 512], bf,
                                        name=f"cu{hp}{qt}{j}", tag="cu", bufs=5)
                    nc.vector.tensor_copy(out=cu[:], in_=cps[0:DK, :])
                    cu_store[(qt, j)] = cu
                recips = norm_pool.tile([33, 512], f32, name=f"rc{hp}_{qt}",
                                        tag="recips", bufs=2)
                nc.vector.reciprocal_approx_fast(out=recips[:], in_=stage[:])
                if hp == NHP - 1 and qt == NQT - 1:
                    # final q-tile: broadcast the reciprocals via a PE
                    # ones-matmul instead of the DRAM round-trip — this
                    # normalization gates the last out-proj groups
                    for j in range(2):
                        bps = psum_c.tile([DK, 512], f32, name=f"bps{j}",
                                          tag="ps512")
                        nc.tensor.matmul(
                            bps[:], ones33[32 * j:32 * j + 1, 0:DK],
                            recips[32 * j:32 * j + 1, :],
                            start=True, stop=True)
                        nc.vector.tensor_mul(
                            out=ctx_sb[hp][64 * j:64 * (j + 1),
                                           512 * qt:512 * (qt + 1)],
                            in0=cu_store[(qt, j)][:], in1=bps[:])
                else:
                    rdram = dram_pool.tile([2, 512], f32, name=f"rd{hp}{qt}",
                                           tag="rd")
                    for j in range(2):
                        nc.sync.dma_start(
                            out=rdram[j:j + 1, :],
                            in_=recips[32 * j:32 * j + 1, :])
                    for j in range(2):
                        bcast = norm_pool.tile(
                            [DK, 512], f32, name=f"b{hp}{qt}{j}",
                            tag="bcast", bufs=2)
                        nc.sync.dma_start(
                            out=bcast[:],
                            in_=rdram[j:j + 1, :].to_broadcast([DK, 512]))
                        nc.vector.tensor_mul(
                            out=ctx_sb[hp][64 * j:64 * (j + 1),
                                           512 * qt:512 * (qt + 1)],
                            in0=cu_store[(qt, j)][:], in1=bcast[:])
                if hp == NHP - 1:
                    # this q-tile's ctx now final across all head pairs
                    for mo in range(NMO):
                        out_queue.append((mo, qt))
            # small-S configs: drain work not absorbed by qt>=2 slots
            for args in pending:
                emit_proj_ntile(args)

        # ---- output projection remainder ----------------------------------
        for mo, nt in out_queue:
            emit_out_group(mo, nt)


# ----------------------------------------------------------------------------
# host side
# ----------------------------------------------------------------------------

def _tile_w(wT, nchunks):
    """[K, M] -> [128, nchunks*M] partition-major bf16."""
    K, M = wT.shape
    assert K == nchunks * 128
    return np.ascontiguousarray(
        wT.reshape(nchunks, 128, M).transpose(1, 0, 2).reshape(128, nchunks * M)
    ).astype(BF16)


def make_in_maps(query, key, value, Wq, bq, Wk, bk, Wv, bv, Wo, S=2048):
    in_maps = []
    for c in range(N_CORES):
        b, g = divmod(c, 2)
        hd = slice(DH * g, DH * (g + 1))
        m = {
            "xq": np.ascontiguousarray(query[:, b, :].T).astype(BF16),
            "xk": np.ascontiguousarray(key[:, b, :].T).astype(BF16),
            "xv": np.ascontiguousarray(value[:, b, :].T).astype(BF16),
            "wq": _tile_w(np.ascontiguousarray(Wq[hd, :].T), D // 128),
            "wk": _tile_w(np.ascontiguousarray(Wk[hd, :].T), D // 128),
            "wv": _tile_w(np.ascontiguousarray(Wv[hd, :].T), D // 128),
            "wo": _tile_w(np.ascontiguousarray(Wo[:, hd].T), DH // 128),
            "bq": bq[hd].reshape(1, DH).astype(BF16),
            "bk": bk[hd].reshape(1, DH).astype(BF16),
            "bv": bv[hd].reshape(1, DH).astype(BF16),
        }
        in_maps.append(m)
    return in_maps


def combine_outputs(results, bo, S=2048):
    out = np.empty((S, 4, D), np.float32)
    for b in range(4):
        pT = results[2 * b]["out"] + results[2 * b + 1]["out"]  # [D, S]
        out[:, b, :] = pT.T
    out += bo.astype(np.float32)[None, None, :]
    return out


_NC_CACHE = {}


def get_program(S=2048, has_bias=False):
    key = (S, has_bias)
    if key not in _NC_CACHE:
        _NC_CACHE[key] = build_program(S, has_bias=has_bias)
    return _NC_CACHE[key]


def kernel(query, key, value, Wq, bq, Wk, bk, Wv, bv, Wo, bo, **_):
    query = np.asarray(query, np.float32)
    key = np.asarray(key, np.float32)
    value = np.asarray(value, np.float32)
    S = query.shape[0]
    has_bias = any(float(np.abs(np.asarray(b)).max()) != 0.0
                   for b in (bq, bk, bv))
    nc = get_program(S, has_bias=has_bias)
    in_maps = make_in_maps(query, key, value,
                           np.asarray(Wq), np.asarray(bq),
                           np.asarray(Wk), np.asarray(bk),
                           np.asarray(Wv), np.asarray(bv),
                           np.asarray(Wo), S=S)
    res = run_bass_kernel_spmd(nc, in_maps, core_ids=list(range(N_CORES)))
    return combine_outputs(res.results, np.asarray(bo), S=S)

